# revision 1
# baseline (speedup 1.0000x reference)
"""Multi-head attention (B=2, L=2048, D=1024, H=16) on 8 TRN2 NeuronCores.

Sharding: batch x head-group. Core c handles batch c//4 and heads
4*(c%4) .. 4*(c%4)+3. Each core:
  - projects its q/k/v slices (transposed activations fed from host),
  - runs flash-style attention in the "S-transposed" layout
    (keys on partitions, queries on free dim) so no on-device transposes
    are ever needed,
  - computes a partial output projection against its Wo column slice.
Host sums the 4 partials per batch.

All matmuls run in float32r (TF32-like, full PE rate at moving dim>=256).
Softmax uses exp without max-subtraction (scores are O(1) by construction);
the attention mask folds into the exp bias, and the softmax denominator
comes for free from a ones-row appended to V.

Emission is software-pipelined: projections are chunked by 512-token
groups and the first two attention groups stream behind the input DMA,
so the ScalarE exp pipeline starts ~15us in instead of after all
projections.
"""
import sys

sys.path.insert(0, "/opt/trn_rl_repo")

import numpy as np
from contextlib import ExitStack

import concourse.bass as bass
import concourse.mybir as mybir
import concourse.tile as tile
from concourse import bacc
from concourse.bass import ts
from concourse.bass_utils import run_bass_kernel_spmd

F32 = mybir.dt.float32
F32R = mybir.dt.float32r
EXP = mybir.ActivationFunctionType.Exp

B = 2
L = 2048
D = 1024
H = 16
DH = 64
HG = 4          # heads per core
NC = 8          # cores
P = 128
DT = D // P     # 8 d-tiles
JT = L // P     # 16 key tiles
IC = L // 512   # 4 query chunks of 512
G = 4           # projection token groups (512 tokens each)

_BUILT = None


def _build():
    nc = bacc.Bacc("TRN2", target_bir_lowering=False, debug=False, num_devices=1)

    xqT_d = nc.dram_tensor("xqT", (D, L), F32R, kind="ExternalInput").ap()
    xkT_d = nc.dram_tensor("xkT", (D, L), F32R, kind="ExternalInput").ap()
    xvT_d = nc.dram_tensor("xvT", (D, L), F32R, kind="ExternalInput").ap()
    wqT_d = nc.dram_tensor("wqT", (D, HG * DH), F32R, kind="ExternalInput").ap()
    wkT_d = nc.dram_tensor("wkT", (D, HG * DH), F32R, kind="ExternalInput").ap()
    wvT_d = nc.dram_tensor("wvT", (D, HG * DH), F32R, kind="ExternalInput").ap()
    woT_d = nc.dram_tensor("woT", (HG * DH, D), F32R, kind="ExternalInput").ap()
    mb_d = nc.dram_tensor("mb", (P, JT), F32, kind="ExternalInput").ap()
    out_d = nc.dram_tensor("partial", (L, D), F32, kind="ExternalOutput").ap()

    marks = []

    def mark(label):
        marks.append((label, int(nc.get_next_instruction_name().split("-")[1])))

    with tile.TileContext(nc) as tc, ExitStack() as ctx:
        perm = ctx.enter_context(tc.tile_pool(name="perm", bufs=1))

        # resident weights
        wq = perm.tile([P, DT, HG * DH], F32R)
        wk = perm.tile([P, DT, HG * DH], F32R)
        wv = perm.tile([P, DT, HG * DH], F32R)
        wo = perm.tile([P, 2, D], F32R)
        mb = perm.tile([P, JT], F32)
        nc.sync.dma_start(wq[:], wqT_d.rearrange("(dt p) m -> p dt m", p=P))
        nc.sync.dma_start(wk[:], wkT_d.rearrange("(dt p) m -> p dt m", p=P))
        nc.sync.dma_start(wv[:], wvT_d.rearrange("(dt p) m -> p dt m", p=P))
        nc.sync.dma_start(mb[:], mb_d[:])
        ones1 = perm.tile([P, 1], F32)
        nc.gpsimd.memset(ones1[:], 1.0)

        QT = [perm.tile([P, 2, 512], F32R, tag=f"QT{g}", name=f"QT{g}") for g in range(G)]
        KT = [perm.tile([P, 2, 512], F32R, tag=f"KT{g}", name=f"KT{g}") for g in range(G)]
        VT = [perm.tile([P, HG * (DH + 1)], F32R, tag=f"VT{j}", name=f"VT{j}") for j in range(JT)]
        OT = [perm.tile([P, 2, 512], F32R, tag=f"OT{g}", name=f"OT{g}") for g in range(G)]

        xpool = ctx.enter_context(tc.tile_pool(name="xg", bufs=4))
        spool = ctx.enter_context(tc.tile_pool(name="spool", bufs=2, space="PSUM"))
        ptpool = ctx.enter_context(tc.tile_pool(name="pt", bufs=3))
        stpool = ctx.enter_context(tc.tile_pool(name="st", bufs=2))
        small = ctx.enter_context(tc.tile_pool(name="small", bufs=2))

        def sslot():
            return spool.tile([P, 1024], F32, tag="s", name="s")

        def proj_group(g, xq_g, xk_g, xv_g):
            # Q and K: out [pair-heads on partitions, 512 tokens]
            for src, wt, dst in ((xq_g, wq, QT[g]), (xk_g, wk, KT[g])):
                ps = sslot()
                for d in range(DT):
                    for p in range(2):
                        nc.tensor.matmul(
                            ps[:, ts(p, 512)], wt[:, d, ts(p, P)], src[:, d, :],
                            start=(d == 0), stop=(d == DT - 1),
                        )
                for p in range(2):
                    nc.vector.tensor_copy(dst[:, p, :], ps[:, ts(p, 512)])
            # V: natural layout [tokens, head dh] + ones column
            for jt in range(4 * g, 4 * g + 4):
                psv = sslot()
                for d in range(DT):
                    nc.tensor.matmul(
                        psv[:, 0:HG * DH],
                        xv_g[:, d, ts(jt % 4, P)], wv[:, d, :],
                        start=(d == 0), stop=(d == DT - 1),
                    )
                vg = VT[jt].rearrange("p (h c) -> p h c", c=DH + 1)
                nc.vector.tensor_copy(
                    vg[:, :, DH:DH + 1], ones1[:, None, :].to_broadcast((P, HG, 1))
                )
                nc.vector.tensor_copy(
                    vg[:, :, 0:DH],
                    psv[:, 0:HG * DH].rearrange("p (h c) -> p h c", c=DH),
                )

        # PV matmuls lag one j behind their exp in the PE stream so the PE
        # never stalls on the current j's exp; norms and output-projection
        # blocks are sprinkled into the NEXT group's j-loop as PE filler.
        PENDING = []   # [(hp, pvs, j, pt)]
        FILLER = []    # deferred closures (norm halves / oproj blocks)

        def flush_pv(keep):
            while len(PENDING) > keep:
                hp, pvs, j, pt = PENDING.pop(0)
                for h01 in range(2):
                    h = 2 * hp + h01
                    nc.tensor.matmul(
                        pvs[h01][:],
                        VT[j][:, h * (DH + 1):(h + 1) * (DH + 1)],
                        pt[:, ts(h01, 512)],
                        start=(j == 0), stop=(j == JT - 1),
                    )

        def attn_jseg(hp, ic, grp, j_range, fill=True):
            for j in j_range:
                ps = sslot()
                nc.tensor.matmul(
                    ps[:, 0:512],
                    KT[j // 4][0:DH, hp, ts(j % 4, P)],
                    QT[ic][0:DH, hp, :],
                    start=True, stop=True,
                )
                nc.tensor.matmul(
                    ps[:, 512:1024],
                    KT[j // 4][DH:P, hp, ts(j % 4, P)],
                    QT[ic][DH:P, hp, :],
                    start=True, stop=True, tile_position=(DH, 0),
                )
                pt = ptpool.tile([P, 1024], F32R, tag="pt", name="pt")
                nc.scalar.activation(
                    pt[:], ps[:], EXP, bias=mb[:, j:j + 1], scale=0.125,
                )
                PENDING.append((hp, grp["pvs"], j, pt))
                flush_pv(1)
                if fill and FILLER:
                    FILLER.pop(0)()

        def attn_norm(hp, ic, grp):
            for h01 in range(2):
                pv = grp["pvs"][h01]
                rec = small.tile([1, 512], F32, tag="rec", name="rec")
                nc.vector.reciprocal(rec[:], pv[DH:DH + 1, :])
                bc = small.tile([DH, 512], F32, tag="bc", name="bc")
                nc.gpsimd.partition_broadcast(bc[:], rec[:])
                nc.vector.tensor_mul(
                    out=OT[ic][ts(h01, DH), hp, :],
                    in0=pv[0:DH, :], in1=bc[:],
                )

        def new_grp(pvpool):
            return {
                "pvs": [
                    pvpool.tile([DH + 1, 512], F32, tag="pvA", name="pvA"),
                    pvpool.tile([DH + 1, 512], F32, tag="pvB", name="pvB"),
                ],
            }

        def make_oproj(auxpool):
            def oproj(tb):
                ic = tb // 4
                st = stpool.tile([P, D], F32, tag="st", name="st")
                for mc in range(2):
                    pso = auxpool.tile([P, 512], F32, tag="pso", name="pso")
                    for kt in range(2):
                        nc.tensor.matmul(
                            pso[:], OT[ic][:, kt, ts(tb % 4, P)],
                            wo[:, kt, ts(mc, 512)],
                            start=(kt == 0), stop=(kt == 1),
                        )
                    nc.vector.tensor_copy(st[:, ts(mc, 512)], pso[:])
                nc.sync.dma_start(out_d[ts(tb, P), :], st[:])
            return oproj

        # ---------- pipelined emission ----------
        # Ramp: DMA + projections stream in 512-token groups; attention
        # groups (0,0) and (0,1) trail them. Dedicated 4-bank pv pool.
        with tc.tile_pool(name="rampv", bufs=2, space="PSUM") as rampv:
            g00 = new_grp(rampv)
            g01 = None
            for g in range(G):
                xq_g = xpool.tile([P, DT, 512], F32R, tag="xg", name="xq_g")
                xk_g = xpool.tile([P, DT, 512], F32R, tag="xg", name="xk_g")
                xv_g = xpool.tile([P, DT, 512], F32R, tag="xg", name="xv_g")
                nc.sync.dma_start(xq_g[:], xqT_d.rearrange("(dt p) t -> p dt t", p=P)[:, :, ts(g, 512)])
                nc.sync.dma_start(xk_g[:], xkT_d.rearrange("(dt p) t -> p dt t", p=P)[:, :, ts(g, 512)])
                nc.sync.dma_start(xv_g[:], xvT_d.rearrange("(dt p) t -> p dt t", p=P)[:, :, ts(g, 512)])
                mark(f"dma{g}")
                proj_group(g, xq_g, xk_g, xv_g)
                mark(f"proj{g}")
                attn_jseg(0, 0, g00, range(4 * g, 4 * g + 4))
                if g >= 1:
                    if g01 is None:
                        g01 = new_grp(rampv)
                    attn_jseg(0, 1, g01, range(4 * (g - 1), 4 * g))
                mark(f"attn_pipe{g}")
            attn_jseg(0, 1, g01, range(12, 16))
            flush_pv(0)
            attn_norm(0, 0, g00)
            attn_norm(0, 1, g01)
            mark("ramp_end")

        nc.sync.dma_start(wo[:], woT_d.rearrange("(kt p) m -> p kt m", p=P))

        # Steady state: 2-bank pv pool + 2-bank oproj pool; norms and output
        # projections fill the next group's j-loop.
        with tc.tile_pool(name="stpv", bufs=1, space="PSUM") as stpv, \
             tc.tile_pool(name="aux", bufs=2, space="PSUM") as auxpool:
            oproj = make_oproj(auxpool)

            def finish_group(hp, ic, grp):
                def _norm():
                    # flush this group's remaining PVs (FIFO head) but not the
                    # already-pending PVs of the group that follows it
                    while any(pvs is grp["pvs"] for _, pvs, _, _ in PENDING):
                        flush_pv(len(PENDING) - 1)
                    attn_norm(hp, ic, grp)
                    if hp == 1:
                        for tb in range(4 * ic, 4 * ic + 4):
                            FILLER.append(lambda tb=tb: oproj(tb))
                FILLER.append(_norm)

            groups = [(0, 2), (0, 3), (1, 0), (1, 1), (1, 2), (1, 3)]
            prev = None
            for hp, ic in groups:
                grp = new_grp(stpv)
                if prev is not None:
                    attn_jseg(*prev)
                    finish_group(prev[0], prev[1], prev[2])
                    mark(f"attn{prev[0]}{prev[1]}")
                prev = (hp, ic, grp, range(JT))
            attn_jseg(*prev)
            finish_group(prev[0], prev[1], prev[2])
            mark("attn_last")
            flush_pv(0)
            while FILLER:
                FILLER.pop(0)()
            mark("drain")

    nc.compile()
    nc._phase_marks = marks
    return nc


def kernel(q, k, v, attention_mask, Wq, Wk, Wv, Wo):
    global _BUILT
    if _BUILT is None:
        _BUILT = _build()
    nc = _BUILT

    q = np.asarray(q, dtype=np.float32)
    k = np.asarray(k, dtype=np.float32)
    v = np.asarray(v, dtype=np.float32)
    Wq = np.asarray(Wq, dtype=np.float32)
    Wk = np.asarray(Wk, dtype=np.float32)
    Wv = np.asarray(Wv, dtype=np.float32)
    Wo = np.asarray(Wo, dtype=np.float32)
    mask = np.asarray(attention_mask)

    xT = {}
    for b in range(B):
        xT[("q", b)] = np.ascontiguousarray(q[b].T)
        xT[("k", b)] = np.ascontiguousarray(k[b].T)
        xT[("v", b)] = np.ascontiguousarray(v[b].T)

    in_maps = []
    for c in range(NC):
        b, hg = c // HG, c % HG
        rows = slice(hg * HG * DH, (hg + 1) * HG * DH)
        mbn = np.where(mask[b] == 0, np.float32(-1e9), np.float32(0.0))
        in_maps.append({
            "xqT": xT[("q", b)],
            "xkT": xT[("k", b)],
            "xvT": xT[("v", b)],
            "wqT": np.ascontiguousarray(Wq[rows].T),
            "wkT": np.ascontiguousarray(Wk[rows].T),
            "wvT": np.ascontiguousarray(Wv[rows].T),
            "woT": np.ascontiguousarray(Wo[:, rows].T),
            "mb": np.ascontiguousarray(mbn.reshape(JT, P).T),
        })

    res = run_bass_kernel_spmd(nc, in_maps, core_ids=list(range(NC)))
    kernel.last_results = res

    out = np.zeros((B, L, D), dtype=np.float64)
    for c in range(NC):
        out[c // HG] += res.results[c]["partial"].astype(np.float64)
    return out.astype(np.float32)

